# revision 9
# baseline (speedup 1.0000x reference)
"""Trainium2 Bass kernel for nn_CoarseMatching (sinkhorn coarse matching).

Computes, for feat_c0 [N,L,C], feat_c1 [N,S,C]:
  sim  = feat_c0 @ feat_c1^T                      [N,L,S]
  conf = softmax_over_{L+1}( [sim; score] / T )   [N,L+1,S]  (dustbin row)
  mask = (conf>THR) & border & (conf==colmax)     [N,L,S]   (bool)
  mconf = where(mask, conf, 0)                    [N,L,S]

Strategy: shard S across 8 cores (512 cols each). Per (n, s-tile of 128):
compute exp(10*sim) in [s_partition, l_free] layout (softmax reductions are
free-dim), fused sum on ScalarE, Max8+argmax on VectorE, normalize in place,
PE-transpose to output layout, contiguous DMA out. mask/mconf are written as
zero-fill + per-column argmax scatter (at most one nonzero per column).
Matmul runs as bf16 hi/lo 3-term decomposition (~5e-6 abs error on sim).
"""

import numpy as np
import ml_dtypes

import concourse.bass as bass
import concourse.bacc as bacc
import concourse.mybir as mybir
import concourse.tile as tile
from concourse.bass import IndirectOffsetOnAxis
from concourse.bass_utils import run_bass_kernel_spmd
from concourse.masks import make_identity
from concourse.tile_rust import add_dep_helper

TEMPERATURE = 0.1
THR = 0.2
BORDER_RM = 2

N_CORES = 8
N, L, C, S = 4, 4800, 256, 4096
SC = S // N_CORES          # 512 columns per core
N_ST = SC // 128           # 4 s-tiles of 128 partitions per core
KC = C // 128              # 2 contraction chunks
LCH = 480                  # l-chunk for PSUM (<=512 fp32/bank)
N_LCH = L // LCH           # 10
N_TCH = (L + 127) // 128   # 38 transpose chunks (37*128 + 64)

_cache = {}
RUN_KWARGS = {}      # test harness may set {"trace": True}
LAST_RESULTS = None  # BassKernelResults of the most recent run


def _build_program(score_f, dt_mm):
    """Build+compile the SPMD single-core program. score_f folded as consts."""
    inv_t = 1.0 / TEMPERATURE
    exp_bin = float(np.exp(np.float32(score_f) * np.float32(inv_t)))

    nc = bacc.Bacc("TRN2", target_bir_lowering=False, debug=False,
                   num_devices=N_CORES)
    dt = mybir.dt
    f32, u8, u32 = dt.float32, dt.uint8, dt.uint32

    f0_d = nc.dram_tensor("f0t", [N, 128, KC, 2, L], dt_mm, kind="ExternalInput").ap()
    f1_d = nc.dram_tensor("f1t", [N, 128, KC, 2, SC], dt_mm, kind="ExternalInput").ap()
    bord_d = nc.dram_tensor("border", [L, 1], f32, kind="ExternalInput").ap()
    sloc_d = nc.dram_tensor("sloc", [128, N_ST], dt.int32, kind="ExternalInput").ap()

    conf_d = nc.dram_tensor("conf", [N, L + 1, SC], f32, kind="ExternalOutput").ap()
    mask_d = nc.dram_tensor("mask", [N, L, SC], u8, kind="ExternalOutput").ap()
    mconf_d = nc.dram_tensor("mconf", [N, L, SC], f32, kind="ExternalOutput").ap()

    mask_flat = mask_d.rearrange("n l s -> n (l s)")
    mconf_flat = mconf_d.rearrange("n l s -> n (l s)")
    # zero-offset flat views for indirect scatters (n handled via element_offset)
    mask_sc = mask_d.rearrange("n l s -> (n l s)")[:, None]
    mconf_sc = mconf_d.rearrange("n l s -> (n l s)")[:, None]
    conf_sc = conf_d.rearrange("n l s -> (n l s)")[:, None]

    with tile.TileContext(nc) as tc:
        with (
            tc.tile_pool(name="consts", bufs=1) as consts,
            tc.tile_pool(name="f0", bufs=2) as f0p,
            tc.tile_pool(name="f1", bufs=2) as f1p,
            tc.tile_pool(name="conf", bufs=4) as confp,
            tc.tile_pool(name="small", bufs=8) as smallp,
            tc.tile_pool(name="strip", bufs=3) as stripp,
            tc.tile_pool(name="mm", bufs=3, space="PSUM") as mmp,
            tc.tile_pool(name="tp", bufs=2, space="PSUM") as tpp,
        ):
            ident = consts.tile([128, 128], f32)
            make_identity(nc, ident[:])
            zf = consts.tile([128, 2048], f32)
            nc.vector.memset(zf[:], 0.0)
            zb = consts.tile([128, 2048], u8)
            nc.vector.memset(zb[:], 0)
            sloc = consts.tile([128, N_ST], dt.int32)
            nc.sync.dma_start(out=sloc[:], in_=sloc_d[:])

            # DRAM zero-fill for mask/mconf (contiguous 1MB-ish chunks).
            zero_insts = [[] for _ in range(N)]
            FL = L * SC  # 2457600 elements per n, = 128 * 19200
            PERP = FL // 128  # 19200
            for n in range(N):
                for x0 in range(0, PERP, 2048):
                    xw = min(2048, PERP - x0)
                    mc = mconf_flat[n].rearrange("(p x) -> p x", p=128)
                    i1 = nc.sync.dma_start(out=mc[:, x0:x0 + xw], in_=zf[:, :xw])
                    mk = mask_flat[n].rearrange("(p x) -> p x", p=128)
                    i2 = nc.sync.dma_start(out=mk[:, x0:x0 + xw], in_=zb[:, :xw])
                    zero_insts[n] += [i1, i2]

            for n in range(N):
                f0 = f0p.tile([128, KC, 2, L], dt_mm, tag="f0")
                nc.sync.dma_start(out=f0[:], in_=f0_d[n])
                f1 = f1p.tile([128, KC, 2, SC], dt_mm, tag="f1")
                nc.sync.dma_start(out=f1[:], in_=f1_d[n])

                conf_ts = []
                for st in range(N_ST):
                    cf = confp.tile([128, L], f32, tag="conf")
                    stats = smallp.tile([128, N_LCH], f32, tag="stats")
                    for lc in range(N_LCH):
                        l0 = lc * LCH
                        ps = mmp.tile([128, LCH], f32, space="PSUM", tag="mm")
                        first = True
                        for (ta, tb) in ((0, 0), (0, 1), (1, 0)):
                            for k in range(KC):
                                nc.tensor.matmul(
                                    ps[:],
                                    f1[:, k, ta, st * 128:(st + 1) * 128],
                                    f0[:, k, tb, l0:l0 + LCH],
                                    start=first, stop=(ta, tb, k) == (1, 0, KC - 1),
                                )
                                first = False
                        nc.scalar.activation(
                            cf[:, l0:l0 + LCH], ps[:],
                            mybir.ActivationFunctionType.Exp,
                            scale=inv_t,
                            accum_out=stats[:, lc:lc + 1],
                        )

                    # z, recip, max, argmax
                    z = smallp.tile([128, 1], f32, tag="z")
                    nc.vector.tensor_reduce(
                        out=z[:], in_=stats[:], axis=mybir.AxisListType.X,
                        op=mybir.AluOpType.add,
                    )
                    z2 = smallp.tile([128, 1], f32, tag="z2")
                    nc.vector.tensor_scalar_add(z2[:], z[:], exp_bin)
                    rz = smallp.tile([128, 1], f32, tag="rz")
                    nc.vector.reciprocal(rz[:], z2[:])

                    vmax = smallp.tile([128, 8], f32, tag="vmax")
                    nc.vector.max(vmax[:], cf[:])
                    vidx = smallp.tile([128, 8], u32, tag="vidx")
                    nc.vector.max_index(vidx[:], vmax[:], cf[:])

                    # normalize in place -> conf
                    nc.vector.tensor_scalar_mul(cf[:], cf[:], rz[:, :1])
                    conf_ts.append(cf)

                    # ---- mask/mconf scatter values ----
                    cmax = smallp.tile([128, 1], f32, tag="cmax")
                    nc.vector.tensor_tensor(
                        out=cmax[:], in0=vmax[:, :1], in1=rz[:],
                        op=mybir.AluOpType.mult)
                    dub = smallp.tile([128, 1], f32, tag="dub")
                    nc.vector.tensor_scalar_mul(dub[:], rz[:], exp_bin)
                    # border gather
                    bg = smallp.tile([128, 1], f32, tag="bg")
                    gi = nc.gpsimd.indirect_dma_start(
                        out=bg[:], out_offset=None,
                        in_=bord_d[:],
                        in_offset=IndirectOffsetOnAxis(ap=vidx[:, :1], axis=0))
                    t1 = smallp.tile([128, 1], f32, tag="t1")
                    nc.vector.tensor_scalar(
                        out=t1[:], in0=cmax[:], scalar1=THR, scalar2=None,
                        op0=mybir.AluOpType.is_gt)
                    t2 = smallp.tile([128, 1], f32, tag="t2")
                    nc.vector.tensor_tensor(
                        out=t2[:], in0=cmax[:], in1=dub[:],
                        op=mybir.AluOpType.is_ge)
                    keep = smallp.tile([128, 1], f32, tag="keep")
                    nc.vector.tensor_tensor(
                        out=keep[:], in0=t1[:], in1=t2[:],
                        op=mybir.AluOpType.mult)
                    nc.vector.tensor_tensor(
                        out=keep[:], in0=keep[:], in1=bg[:],
                        op=mybir.AluOpType.mult)
                    smv = smallp.tile([128, 1], f32, tag="smv")
                    nc.vector.tensor_tensor(
                        out=smv[:], in0=cmax[:], in1=keep[:],
                        op=mybir.AluOpType.mult)
                    ku8 = smallp.tile([128, 1], u8, tag="ku8")
                    nc.vector.tensor_copy(ku8[:], keep[:])

                    # offsets: idx = l* * SC + (st*128 + p)
                    lf = smallp.tile([128, 1], f32, tag="lf")
                    nc.vector.tensor_copy(lf[:], vidx[:, :1])
                    nc.vector.tensor_scalar_mul(lf[:], lf[:], float(SC))
                    lu = smallp.tile([128, 1], u32, tag="lu")
                    nc.vector.tensor_copy(lu[:], lf[:])
                    idx = smallp.tile([128, 1], u32, tag="idx")
                    nc.vector.tensor_tensor(
                        out=idx[:], in0=lu[:], in1=sloc[:, st:st + 1].bitcast(u32),
                        op=mybir.AluOpType.add)

                    s1 = nc.gpsimd.indirect_dma_start(
                        out=mconf_sc, out_offset=IndirectOffsetOnAxis(
                            ap=idx[:, :1], axis=0),
                        in_=smv[:], in_offset=None,
                        element_offset=n * L * SC)
                    s2 = nc.gpsimd.indirect_dma_start(
                        out=mask_sc, out_offset=IndirectOffsetOnAxis(
                            ap=idx[:, :1], axis=0),
                        in_=ku8[:], in_offset=None,
                        element_offset=n * L * SC)
                    for zi in zero_insts[n]:
                        add_dep_helper(s1.ins, zi.ins, sync=True, reason="zerofill<scatter")
                        add_dep_helper(s2.ins, zi.ins, sync=True, reason="zerofill<scatter")

                    # dustbin conf row: conf[n, L, st*128+p] = exp_bin * rz
                    idx2 = smallp.tile([128, 1], u32, tag="idx2")
                    nc.vector.tensor_copy(idx2[:], sloc[:, st:st + 1].bitcast(u32))
                    nc.gpsimd.indirect_dma_start(
                        out=conf_sc,
                        out_offset=IndirectOffsetOnAxis(ap=idx2[:, :1], axis=0),
                        in_=dub[:], in_offset=None,
                        element_offset=n * (L + 1) * SC + L * SC)

                # ---- transpose + write conf rows ----
                for tcH in range(N_TCH):
                    l0 = tcH * 128
                    lw = min(128, L - l0)
                    pst = tpp.tile([128, SC], f32, space="PSUM", tag="tp")
                    for st in range(N_ST):
                        nc.tensor.transpose(
                            out=pst[:lw, st * 128:(st + 1) * 128],
                            in_=conf_ts[st][:, l0:l0 + lw],
                            identity=ident[:])
                    strip = stripp.tile([128, SC], f32, tag="strip")
                    nc.scalar.copy(strip[:lw], pst[:lw])
                    nc.sync.dma_start(
                        out=conf_d[n, l0:l0 + lw, :], in_=strip[:lw])

    nc.compile()
    return nc


def _split_bf16(x):
    hi = x.astype(ml_dtypes.bfloat16)
    lo = (x - hi.astype(np.float32)).astype(ml_dtypes.bfloat16)
    return hi, lo


def kernel(feat_c0, feat_c1, score, hc, wc):
    feat_c0 = np.asarray(feat_c0, dtype=np.float32)
    feat_c1 = np.asarray(feat_c1, dtype=np.float32)
    score_f = float(np.asarray(score))
    hc_i, wc_i = int(np.asarray(hc)), int(np.asarray(wc))
    assert feat_c0.shape == (N, L, C) and feat_c1.shape == (N, S, C)
    assert hc_i * wc_i == L

    key = (score_f, hc_i, wc_i)
    if key not in _cache:
        _cache[key] = _build_program(score_f, mybir.dt.bfloat16)
    nc = _cache[key]

    # host-side prep: transposed [C-part, kchunk, hi/lo, ...] bf16 layouts
    f0t = np.ascontiguousarray(feat_c0.transpose(0, 2, 1))        # [N, C, L]
    f1t = np.ascontiguousarray(feat_c1.transpose(0, 2, 1))        # [N, C, S]
    f0h, f0l = _split_bf16(f0t)
    f1h, f1l = _split_bf16(f1t)

    def pack(h, lo, last):
        # [N, C, X] -> [N, 128, KC, 2, X]
        out = np.empty((N, 128, KC, 2, last), dtype=ml_dtypes.bfloat16)
        for k in range(KC):
            out[:, :, k, 0, :] = h[:, k * 128:(k + 1) * 128, :]
            out[:, :, k, 1, :] = lo[:, k * 128:(k + 1) * 128, :]
        return out

    f0_packed = pack(f0h, f0l, L)

    rows = np.arange(hc_i)
    cols = np.arange(wc_i)
    vr = (rows >= BORDER_RM) & (rows < hc_i - BORDER_RM)
    vc = (cols >= BORDER_RM) & (cols < wc_i - BORDER_RM)
    border = (vr[:, None] & vc[None, :]).reshape(L, 1).astype(np.float32)

    sloc = (np.arange(SC, dtype=np.int32).reshape(N_ST, 128).T).copy()  # [128, N_ST]

    in_maps = []
    for c in range(N_CORES):
        sl = slice(c * SC, (c + 1) * SC)
        in_maps.append({
            "f0t": f0_packed,
            "f1t": pack(f1h[:, :, sl], f1l[:, :, sl], SC),
            "border": border,
            "sloc": sloc,
        })

    global LAST_RESULTS
    res = run_bass_kernel_spmd(nc, in_maps, list(range(N_CORES)), **RUN_KWARGS)
    LAST_RESULTS = res

    conf = np.concatenate([res.results[i]["conf"] for i in range(N_CORES)], axis=2)
    mask = np.concatenate([res.results[i]["mask"] for i in range(N_CORES)], axis=2)
    mconf = np.concatenate([res.results[i]["mconf"] for i in range(N_CORES)], axis=2)
    return conf, mask.astype(bool), mconf


# revision 17
# speedup vs baseline: 65887.9142x; 65887.9142x over previous
"""Trainium2 Bass kernel for nn_CoarseMatching (sinkhorn coarse matching).

Computes, for feat_c0 [N,L,C], feat_c1 [N,S,C]:
  sim  = feat_c0 @ feat_c1^T                      [N,L,S]
  conf = softmax_over_{L+1}( [sim; score] / T )   [N,L+1,S]  (dustbin row)
  mask = (conf>THR) & border & (conf==colmax)     [N,L,S]   (bool)
  mconf = where(mask, conf, 0)                    [N,L,S]

Strategy: shard S across 8 cores (512 cols each). Per (n, s-tile of 128):
compute exp(10*sim) in [s_partition, l_free] layout (softmax reductions are
free-dim), fused sum on ScalarE, Max8+argmax on VectorE, normalize in place,
PE-transpose to output layout, contiguous DMA out. mask/mconf are written as
zero-fill + per-column argmax scatter (at most one nonzero per column).
Matmul runs as bf16 hi/lo 3-term decomposition (~5e-6 abs error on sim).
"""

import numpy as np
import ml_dtypes

import concourse.bass as bass
import concourse.bacc as bacc
import concourse.mybir as mybir
import concourse.tile as tile
from concourse.bass import IndirectOffsetOnAxis
from concourse.bass_utils import run_bass_kernel_spmd
from concourse.masks import make_identity
from concourse.tile_rust import add_dep_helper

TEMPERATURE = 0.1
THR = 0.2
BORDER_RM = 2

N_CORES = 8
N, L, C, S = 4, 4800, 256, 4096
SC = S // N_CORES          # 512 columns per core
N_ST = SC // 128           # 4 s-tiles of 128 partitions per core
KC = C // 128              # 2 contraction chunks
LCH = 480                  # l-chunk for PSUM (<=512 fp32/bank)
N_LCH = L // LCH           # 10
N_TCH = (L + 127) // 128   # 38 transpose chunks (37*128 + 64)

_cache = {}
RUN_KWARGS = {}      # test harness may set {"trace": True}
LAST_RESULTS = None  # BassKernelResults of the most recent run


SKIP_SCATTERS = False  # timeline-sim probe only (cost model misprices them)


def _build_program(score_f, dt_mm):
    """Build+compile the SPMD single-core program. score_f folded as consts."""
    inv_t = 1.0 / TEMPERATURE
    exp_bin = float(np.exp(np.float32(score_f) * np.float32(inv_t)))

    nc = bacc.Bacc("TRN2", target_bir_lowering=False, debug=False,
                   num_devices=N_CORES)
    dt = mybir.dt
    f32, u8, u32 = dt.float32, dt.uint8, dt.uint32

    f0_d = nc.dram_tensor("f0t", [N, 128, KC, 2, L], dt_mm, kind="ExternalInput").ap()
    f1_d = nc.dram_tensor("f1t", [N, 128, KC, 2, SC], dt_mm, kind="ExternalInput").ap()
    bord_d = nc.dram_tensor("border", [L, 1], f32, kind="ExternalInput").ap()
    sloc_d = nc.dram_tensor("sloc", [128, N_ST], dt.int32, kind="ExternalInput").ap()

    conf_d = nc.dram_tensor("conf", [N, L + 1, SC], f32, kind="ExternalOutput").ap()
    mask_d = nc.dram_tensor("mask", [N, L, SC], u8, kind="ExternalOutput").ap()
    mconf_d = nc.dram_tensor("mconf", [N, L, SC], f32, kind="ExternalOutput").ap()

    mask_flat = mask_d.rearrange("n l s -> n (l s)")
    mconf_flat = mconf_d.rearrange("n l s -> n (l s)")
    # zero-offset flat views for indirect scatters (n handled via element_offset)
    mask_sc = mask_d.rearrange("n l s -> (n l s)")[:, None]
    mconf_sc = mconf_d.rearrange("n l s -> (n l s)")[:, None]
    conf_sc = conf_d.rearrange("n l s -> (n l s)")[:, None]

    with tile.TileContext(nc) as tc:
        with (
            tc.tile_pool(name="consts", bufs=1) as consts,
            tc.tile_pool(name="f0", bufs=2) as f0p,
            tc.tile_pool(name="f1", bufs=2) as f1p,
            tc.tile_pool(name="conf", bufs=4) as confp,
            tc.tile_pool(name="small", bufs=8) as smallp,
            tc.tile_pool(name="strip", bufs=4) as stripp,
            tc.tile_pool(name="mm", bufs=4, space="PSUM") as mmp,
            tc.tile_pool(name="tp", bufs=4, space="PSUM") as tpp,
        ):
            ident = consts.tile([128, 128], f32)
            make_identity(nc, ident[:])
            zf = consts.tile([128, 2048], f32)
            nc.vector.memset(zf[:], 0.0)
            zb = consts.tile([128, 2048], u8)
            nc.vector.memset(zb[:], 0)
            sloc = consts.tile([128, N_ST], dt.int32)
            nc.sync.dma_start(out=sloc[:], in_=sloc_d[:])

            # DRAM zero-fill for mask/mconf (contiguous 1MB-ish chunks).
            # Issued on the ACT HWDGE ring (nc.scalar) so they don't block
            # the SP ring feeding the compute pipeline; emitted inside the
            # n loop so they overlap compute instead of serializing upfront.
            zero_insts = [[] for _ in range(N)]
            FL = L * SC  # 2457600 elements per n, = 128 * 19200
            PERP = FL // 128  # 19200

            for n in range(N):
                f0 = f0p.tile([128, KC, 2, L], dt_mm, tag="f0")
                nc.sync.dma_start(out=f0[:], in_=f0_d[n])
                f1 = f1p.tile([128, KC, 2, SC], dt_mm, tag="f1")
                nc.sync.dma_start(out=f1[:], in_=f1_d[n])

                for x0 in range(0, PERP, 2048):
                    xw = min(2048, PERP - x0)
                    mc = mconf_flat[n].rearrange("(p x) -> p x", p=128)
                    i1 = nc.scalar.dma_start(out=mc[:, x0:x0 + xw], in_=zf[:, :xw])
                    mk = mask_flat[n].rearrange("(p x) -> p x", p=128)
                    i2 = nc.scalar.dma_start(out=mk[:, x0:x0 + xw], in_=zb[:, :xw])
                    zero_insts[n] += [i1, i2]

                for st in range(N_ST):
                    cf = confp.tile([128, L], f32, tag="conf")
                    stats = smallp.tile([128, N_LCH], f32, tag="stats")
                    for lc in range(N_LCH):
                        l0 = lc * LCH
                        ps = mmp.tile([128, LCH], f32, space="PSUM", tag="mm")
                        first = True
                        for (ta, tb) in ((0, 0), (0, 1), (1, 0)):
                            for k in range(KC):
                                nc.tensor.matmul(
                                    ps[:],
                                    f1[:, k, ta, st * 128:(st + 1) * 128],
                                    f0[:, k, tb, l0:l0 + LCH],
                                    start=first, stop=(ta, tb, k) == (1, 0, KC - 1),
                                )
                                first = False
                        nc.scalar.activation(
                            cf[:, l0:l0 + LCH], ps[:],
                            mybir.ActivationFunctionType.Exp,
                            scale=inv_t,
                            accum_out=stats[:, lc:lc + 1],
                        )

                    # z, recip, max, argmax
                    z = smallp.tile([128, 1], f32, tag="z")
                    nc.vector.tensor_reduce(
                        out=z[:], in_=stats[:], axis=mybir.AxisListType.X,
                        op=mybir.AluOpType.add,
                    )
                    z2 = smallp.tile([128, 1], f32, tag="z2")
                    nc.vector.tensor_scalar_add(z2[:], z[:], exp_bin)
                    rz = smallp.tile([128, 1], f32, tag="rz")
                    nc.vector.reciprocal(rz[:], z2[:])

                    vmax = smallp.tile([128, 8], f32, tag="vmax")
                    nc.vector.max(vmax[:], cf[:])
                    vidx = smallp.tile([128, 8], u32, tag="vidx")
                    nc.vector.max_index(vidx[:], vmax[:], cf[:])

                    # normalize in place -> conf
                    nc.vector.tensor_scalar_mul(cf[:], cf[:], rz[:, :1])

                    # transpose + write conf rows for this s-tile. Blocks of
                    # [128l x 128s] are staged 8-at-a-time into one SBUF
                    # buffer so each DMA is ~256KB (HWDGE overhead amortized)
                    # while keeping everything per-stile pipelined.
                    NFULL = L // 128  # 37 full l-chunks
                    for g0 in range(0, NFULL, 8):
                        gn = min(8, NFULL - g0)
                        blkbuf = stripp.tile([128, 8, 128], f32, tag="strip")
                        for q0 in range(0, gn, 4):
                            qn = min(4, gn - q0)
                            pst = tpp.tile([128, 512], f32, space="PSUM",
                                           tag="tp")
                            for j in range(qn):
                                l0 = (g0 + q0 + j) * 128
                                nc.tensor.transpose(
                                    out=pst[:, j * 128:(j + 1) * 128],
                                    in_=cf[:, l0:l0 + 128],
                                    identity=ident[:])
                            nc.scalar.copy(
                                blkbuf[:, q0:q0 + qn, :],
                                pst[:, :qn * 128])
                        rows = gn * 128
                        out_ap = conf_d[n, g0 * 128:g0 * 128 + rows,
                                        st * 128:(st + 1) * 128]
                        nc.sync.dma_start(
                            out=out_ap.rearrange("(c p) s -> p c s", p=128),
                            in_=blkbuf[:, :gn, :])
                    # tail chunk (64 rows)
                    l0 = NFULL * 128
                    lw = L - l0  # 64
                    pst = tpp.tile([128, 512], f32, space="PSUM", tag="tp")
                    nc.tensor.transpose(
                        out=pst[:lw, :128], in_=cf[:, l0:l0 + lw],
                        identity=ident[:])
                    blk = stripp.tile([128, 8, 128], f32, tag="strip")
                    nc.scalar.copy(blk[:lw, 0, :], pst[:lw, :128])
                    nc.sync.dma_start(
                        out=conf_d[n, l0:l0 + lw, st * 128:(st + 1) * 128],
                        in_=blk[:lw, 0, :])

                    # ---- mask/mconf scatter values ----
                    cmax = smallp.tile([128, 1], f32, tag="cmax")
                    nc.vector.tensor_tensor(
                        out=cmax[:], in0=vmax[:, :1], in1=rz[:],
                        op=mybir.AluOpType.mult)
                    dub = smallp.tile([128, 1], f32, tag="dub")
                    nc.vector.tensor_scalar_mul(dub[:], rz[:], exp_bin)
                    # border gather
                    bg = smallp.tile([128, 1], f32, tag="bg")
                    gi = nc.gpsimd.indirect_dma_start(
                        out=bg[:], out_offset=None,
                        in_=bord_d[:],
                        in_offset=IndirectOffsetOnAxis(ap=vidx[:, :1], axis=0))
                    t1 = smallp.tile([128, 1], f32, tag="t1")
                    nc.vector.tensor_scalar(
                        out=t1[:], in0=cmax[:], scalar1=THR, scalar2=None,
                        op0=mybir.AluOpType.is_gt)
                    t2 = smallp.tile([128, 1], f32, tag="t2")
                    nc.vector.tensor_tensor(
                        out=t2[:], in0=cmax[:], in1=dub[:],
                        op=mybir.AluOpType.is_ge)
                    keep = smallp.tile([128, 1], f32, tag="keep")
                    nc.vector.tensor_tensor(
                        out=keep[:], in0=t1[:], in1=t2[:],
                        op=mybir.AluOpType.mult)
                    nc.vector.tensor_tensor(
                        out=keep[:], in0=keep[:], in1=bg[:],
                        op=mybir.AluOpType.mult)
                    smv = smallp.tile([128, 1], f32, tag="smv")
                    nc.vector.tensor_tensor(
                        out=smv[:], in0=cmax[:], in1=keep[:],
                        op=mybir.AluOpType.mult)
                    ku8 = smallp.tile([128, 1], u8, tag="ku8")
                    nc.vector.tensor_copy(ku8[:], keep[:])

                    # offsets: idx = l* * SC + (st*128 + p)
                    lf = smallp.tile([128, 1], f32, tag="lf")
                    nc.vector.tensor_copy(lf[:], vidx[:, :1])
                    nc.vector.tensor_scalar_mul(lf[:], lf[:], float(SC))
                    lu = smallp.tile([128, 1], u32, tag="lu")
                    nc.vector.tensor_copy(lu[:], lf[:])
                    idx = smallp.tile([128, 1], u32, tag="idx")
                    nc.vector.tensor_tensor(
                        out=idx[:], in0=lu[:], in1=sloc[:, st:st + 1].bitcast(u32),
                        op=mybir.AluOpType.add)

                    if not SKIP_SCATTERS:
                        s1 = nc.gpsimd.indirect_dma_start(
                            out=mconf_sc, out_offset=IndirectOffsetOnAxis(
                                ap=idx[:, :1], axis=0),
                            in_=smv[:], in_offset=None,
                            element_offset=n * L * SC)
                        s2 = nc.gpsimd.indirect_dma_start(
                            out=mask_sc, out_offset=IndirectOffsetOnAxis(
                                ap=idx[:, :1], axis=0),
                            in_=ku8[:], in_offset=None,
                            element_offset=n * L * SC)
                        for zi in zero_insts[n]:
                            add_dep_helper(s1.ins, zi.ins, sync=True, reason="zerofill<scatter")
                            add_dep_helper(s2.ins, zi.ins, sync=True, reason="zerofill<scatter")

                    # dustbin conf row: conf[n, L, st*128+p] = exp_bin * rz
                    idx2 = smallp.tile([128, 1], u32, tag="idx2")
                    nc.vector.tensor_copy(idx2[:], sloc[:, st:st + 1].bitcast(u32))
                    if not SKIP_SCATTERS:
                        nc.gpsimd.indirect_dma_start(
                            out=conf_sc,
                            out_offset=IndirectOffsetOnAxis(ap=idx2[:, :1], axis=0),
                            in_=dub[:], in_offset=None,
                            element_offset=n * (L + 1) * SC + L * SC)


    nc.compile()
    return nc


def _split_bf16(x):
    hi = x.astype(ml_dtypes.bfloat16)
    lo = (x - hi.astype(np.float32)).astype(ml_dtypes.bfloat16)
    return hi, lo


def kernel(feat_c0, feat_c1, score, hc, wc):
    feat_c0 = np.asarray(feat_c0, dtype=np.float32)
    feat_c1 = np.asarray(feat_c1, dtype=np.float32)
    score_f = float(np.asarray(score))
    hc_i, wc_i = int(np.asarray(hc)), int(np.asarray(wc))
    assert feat_c0.shape == (N, L, C) and feat_c1.shape == (N, S, C)
    assert hc_i * wc_i == L

    key = (score_f, hc_i, wc_i)
    if key not in _cache:
        _cache[key] = _build_program(score_f, mybir.dt.bfloat16)
    nc = _cache[key]

    # host-side prep: transposed [C-part, kchunk, hi/lo, ...] bf16 layouts
    f0t = np.ascontiguousarray(feat_c0.transpose(0, 2, 1))        # [N, C, L]
    f1t = np.ascontiguousarray(feat_c1.transpose(0, 2, 1))        # [N, C, S]
    f0h, f0l = _split_bf16(f0t)
    f1h, f1l = _split_bf16(f1t)

    def pack(h, lo, last):
        # [N, C, X] -> [N, 128, KC, 2, X]
        out = np.empty((N, 128, KC, 2, last), dtype=ml_dtypes.bfloat16)
        for k in range(KC):
            out[:, :, k, 0, :] = h[:, k * 128:(k + 1) * 128, :]
            out[:, :, k, 1, :] = lo[:, k * 128:(k + 1) * 128, :]
        return out

    f0_packed = pack(f0h, f0l, L)

    rows = np.arange(hc_i)
    cols = np.arange(wc_i)
    vr = (rows >= BORDER_RM) & (rows < hc_i - BORDER_RM)
    vc = (cols >= BORDER_RM) & (cols < wc_i - BORDER_RM)
    border = (vr[:, None] & vc[None, :]).reshape(L, 1).astype(np.float32)

    sloc = (np.arange(SC, dtype=np.int32).reshape(N_ST, 128).T).copy()  # [128, N_ST]

    in_maps = []
    for c in range(N_CORES):
        sl = slice(c * SC, (c + 1) * SC)
        in_maps.append({
            "f0t": f0_packed,
            "f1t": pack(f1h[:, :, sl], f1l[:, :, sl], SC),
            "border": border,
            "sloc": sloc,
        })

    global LAST_RESULTS
    res = run_bass_kernel_spmd(nc, in_maps, list(range(N_CORES)), **RUN_KWARGS)
    LAST_RESULTS = res

    conf = np.concatenate([res.results[i]["conf"] for i in range(N_CORES)], axis=2)
    mask = np.concatenate([res.results[i]["mask"] for i in range(N_CORES)], axis=2)
    mconf = np.concatenate([res.results[i]["mconf"] for i in range(N_CORES)], axis=2)
    return conf, mask.astype(bool), mconf
